# revision 9
# baseline (speedup 1.0000x reference)
"""Trainium2 Bass kernel for a 2-layer LSTM text classifier.

Model (see original nn.Module): embedding lookup -> 2-layer BasicLSTM
(H=100, T=25) -> dense(128) -> dense(2). Batch 512 is data-parallel
across 8 NeuronCores (64 rows/core); all parameters are replicated.
The embedding gather (pure indexing) happens host-side so only the
5 MB of used rows ship to the devices, pre-transposed to the
feature-major layout the kernel wants.

Device kernel design notes (v2 — latency-tuned):
- Feature-major layout everywhere: [hidden=100 partitions, batch=64
  free]; the recurrence never needs a transpose.
- All four gates go through a single tanh activation per cell:
  sigmoid(x) = (1 + tanh(x/2))/2, with the 1/2 prescale folded into
  the i/f/o weight columns host-side. Cell state is kept doubled
  (C = 2c):
      qh = (tf + 1) * C           # = 4 c sigmoid(f+1)
      ph = (ti + 1) * tj          # = 2 sigmoid(i) tanh(j)
      C' = qh * 0.5 + ph          # = 2 c'
      tc = tanh(0.5 * C')         # = tanh(c')
      h~ = (to + 1) * tc          # = 2 h   (next-layer weights absorb 0.5)
- Software pipeline: phase t computes cell1(t) and cell2(t-1). The
  critical chain is cell1's h-recurrence: z1close mms -> gates tanh
  (ACT) -> qh -> C' (DVE) -> tc (ACT) -> h (DVE). Everything of
  cell2 runs in that chain's engine gaps one phase behind.
- ph rides on GpSimd (off the DVE chain), and cell2's C' also runs
  on GpSimd so cell1's h op never queues behind it on DVE.
- z2's PSUM bank opens with the w2h*h2(t-2) part (ready early) and
  closes with w2x*h1(t-1) (the same sem that starts the chain).
- Biases ride in an extra contraction row: xt carries a row of ones
  at partition 100 (b1 in w1x row 100); h2 tiles carry a ones row
  (b2 in w2h row 100); the fused classifier wf12 = wf1 @ wf2 + bias
  rides the same way.
- All inputs are pre-transposed on the host into [128, N] layouts so
  the loads are plain direct DMAs (contiguous >=1KB per partition
  line) split across the SP/ACT/DVE HWDGE rings in first-use order.
- Matmuls are bf16 accumulating in fp32 PSUM; cell state stays fp32.
"""

import functools
import os
import sys

import numpy as np

for _p in ("/opt/trn_rl_repo", "/root/.axon_site/_ro/trn_rl_repo"):
    if os.path.isdir(_p) and _p not in sys.path:
        sys.path.insert(0, _p)
        break

import ml_dtypes

from concourse import bass, bass2jax, mybir
from concourse.bass_utils import run_bass_kernel_spmd
from concourse.tile import TileContext

# --- BIR sync-wait rebalancer -------------------------------------------
# The walrus build in this image enforces ONE sync-wait command per ISA
# instruction struct, but Tile's semaphore assignment happily emits 2-4
# waits on matmuls/DVE ops at psum-recycle points. Rewrite the BIR before
# walrus: park one matmul wait on the adjacent Ldweights (same in-order
# queue, executes strictly before the matmul) and split any remaining
# excess onto pure-wait EventSemaphore carriers inserted directly before
# the offending instruction on its own queue. Semantics are unchanged --
# every wait still completes before the instruction it guarded.

_WAIT_PASSTHROUGH = {"EventSemaphore", "UnconditionalBranch", "Call",
                     "RegisterMove", "ISA"}


def _rebalance_bir_waits(bir_bytes):
    import orjson
    bir = orjson.loads(bir_bytes)
    n = 0
    for fn in bir["functions"]:
        for blk in fn["blocks"]:
            out = []
            prev = None
            for inst in blk["instructions"]:
                op = inst.get("opcode")
                si = inst.get("sync_info") or {}
                waits = si.get("on_wait") or []
                if op not in _WAIT_PASSTHROUGH and len(waits) > 1:
                    if (op == "Matmult" and prev is not None
                            and prev.get("opcode") == "Ldweights"
                            and not (prev.get("sync_info") or {}).get("on_wait")):
                        tsi = prev.setdefault("sync_info", {})
                        tsi.setdefault("on_wait", []).append(waits.pop(0))
                    while len(waits) > 1:
                        n += 1
                        out.append({
                            "debug": inst.get("debug", 0),
                            "engine": inst["engine"],
                            "ins": [], "outs": [],
                            "name": f"antwait_{n}",
                            "opcode": "EventSemaphore",
                            "sync_info": {"on_update": [],
                                          "on_wait": [waits.pop(0)]},
                        })
                    si["on_wait"] = waits
                out.append(inst)
                prev = inst
            blk["instructions"] = out
    return orjson.dumps(bir)


_orig_compile_bir_kernel = bass2jax.compile_bir_kernel


def _compile_bir_kernel_rebalanced(bir_json, tmpdir, neff_name="file.neff"):
    return _orig_compile_bir_kernel(_rebalance_bir_waits(bir_json), tmpdir,
                                    neff_name=neff_name)


if bass2jax.compile_bir_kernel is not _compile_bir_kernel_rebalanced:
    bass2jax.compile_bir_kernel = _compile_bir_kernel_rebalanced

H = 100          # hidden size
T = 25           # sequence length
B = 512          # total batch
N_CORES = 8
BC = B // N_CORES  # 64 per-core batch
NCLS = 2         # logits
FORGET_BIAS = 1.0

BF16 = ml_dtypes.bfloat16
_DT = mybir.dt
TANH = mybir.ActivationFunctionType.Tanh
ADD = mybir.AluOpType.add
MULT = mybir.AluOpType.mult

# gate slot order in PSUM: [i, f, o, j]; source block order in the
# TF BasicLSTMCell kernel is [i, j, f, o]
SLOT_SRC_BLOCK = (0, 2, 3, 1)
SLOT_PRESCALE = (0.5, 0.5, 0.5, 1.0)  # tanh(x/2) trick for i/f/o, plain tanh for j


def _build_nc():
    nc = bass.Bass()
    # host-pretransposed inputs: partition-major, contiguous per row
    w1d = nc.dram_tensor("w1d", [128, 1024], _DT.bfloat16, kind="ExternalInput")
    w2d = nc.dram_tensor("w2d", [128, 1026], _DT.bfloat16, kind="ExternalInput")
    xtd = nc.dram_tensor("xtd", [128, T * BC], _DT.bfloat16, kind="ExternalInput")
    out_d = nc.dram_tensor("out", [NCLS, BC], _DT.float32, kind="ExternalOutput")

    with TileContext(nc) as tc:
        with tc.tile_pool(name="const", bufs=1) as cpool, \
             tc.tile_pool(name="work", bufs=3) as wpool, \
             tc.tile_pool(name="ps1", bufs=2, space="PSUM") as zpool1, \
             tc.tile_pool(name="ps2", bufs=2, space="PSUM") as zpool2, \
             tc.tile_pool(name="psfc", bufs=1, space="PSUM") as fpool:

            wp = cpool.tile([128, 2052], _DT.bfloat16, tag="wp")
            xt = cpool.tile([128, T * BC], _DT.bfloat16, tag="xt")
            hst = cpool.tile([128, 4 * BC], _DT.bfloat16, tag="hst")
            cst = cpool.tile([128, 2 * BC], _DT.float32, tag="cst")
            scratch = cpool.tile([1, 1], _DT.float32, tag="scratch")
            outs = cpool.tile([NCLS, BC], _DT.float32, tag="outs")

            # weight slices (lhsT layout: partitions = contraction dim)
            def w1x(g):
                return wp[0:H + 1, g * 128:(g + 1) * 128]

            def w1h(g):
                return wp[0:H, 512 + g * 128:512 + (g + 1) * 128]

            def w2x(g):
                return wp[0:H, 1024 + g * 128:1024 + (g + 1) * 128]

            def w2h(g):
                return wp[0:H + 1, 1536 + g * 128:1536 + (g + 1) * 128]

            wf12 = wp[0:H + 1, 2048:2050]

            # recurrent state: ping/pong columns; hst partition 100 is the
            # all-ones bias lane (used by the h2-side w2h matmuls + head)
            h1 = [hst[:, 0:BC], hst[:, BC:2 * BC]]
            h2 = [hst[:, 2 * BC:3 * BC], hst[:, 3 * BC:4 * BC]]
            c1 = cst[:, 0:BC]
            c2 = cst[:, BC:2 * BC]

            # input DMAs, split across HWDGE rings in first-use order
            nc.sync.dma_start(out=wp[:, 0:1024], in_=w1d[:, :])
            nc.scalar.dma_start(out=xt[:, 0:8 * BC], in_=xtd[:, 0:8 * BC])
            nc.scalar.dma_start(out=wp[:, 1024:2050], in_=w2d[:, :])
            nc.sync.dma_start(out=xt[:, 8 * BC:T * BC], in_=xtd[:, 8 * BC:T * BC])

            # state init + tanh table warm-up; VEC is idle this early, and
            # GpSimd rejects partition ranges off its 16-partition grid
            nc.vector.memset(scratch[:, :], 0.0)
            nc.scalar.activation(scratch[:, :], scratch[:, :], TANH)
            nc.vector.memset(cst[:, :], 0.0)
            # partition offsets must be 32-aligned, so build the ones row at
            # partition 100 by filling with 1.0 then zeroing rows 0:100
            nc.vector.memset(hst[:, :], 1.0)
            nc.vector.memset(hst[0:H, :], 0.0)

            def cell(tg, c_st, h_wr, c2_on_pool):
                # gates -> new cell state / hidden, [H, BC] layout. Tiles
                # touched by GpSimd span all 128 partitions (Pool accesses
                # must be 16-partition aligned); rows 100-127 carry garbage
                # that no consumer reads.
                ti, tf = tg[:, 0:64], tg[:, 64:128]
                to, tj = tg[0:H, 128:192], tg[:, 192:256]
                qh = wpool.tile([128, BC], _DT.float32, tag="qh")
                nc.vector.scalar_tensor_tensor(qh[:, :], tf, 1.0, c_st, op0=ADD, op1=MULT)
                # ph on GpSimd (walrus only allows tensor_tensor/tensor_scalar
                # there, so (ti+1)*tj takes two ops)
                ui = wpool.tile([128, BC], _DT.bfloat16, tag="ui")
                nc.gpsimd.tensor_scalar_add(ui[:, :], ti, 1.0)
                ph = wpool.tile([128, BC], _DT.bfloat16, tag="ph")
                nc.gpsimd.tensor_tensor(ph[:, :], ui[:, :], tj, op=MULT)
                nc.vector.scalar_tensor_tensor(c_st, qh[:, :], 0.5, ph[:, :], op0=MULT, op1=ADD)
                tcg = wpool.tile([H, BC], _DT.bfloat16, tag="tc")
                nc.scalar.activation(tcg[:, :], c_st[0:H, :], TANH, scale=0.5)
                nc.vector.scalar_tensor_tensor(h_wr[0:H, :], to, 1.0, tcg[:, :], op0=ADD, op1=MULT)

            # prologue: open z1(0) with the x-part (h1(-1)=0, so this also
            # closes it -- the recurrent term is exactly zero at t=0)
            z1 = zpool1.tile([128, 512], _DT.float32, tag="z1")
            for g in range(4):
                nc.tensor.matmul(z1[0:128, g * 64:(g + 1) * 64],
                                 lhsT=w1x(g), rhs=xt[0:H + 1, 0:BC],
                                 start=(g == 0), stop=(g == 3))

            z2 = None
            for t in range(T):
                rd, wr = (t + 1) % 2, t % 2
                # close z1(t) with the recurrent part (the chain head)
                if t > 0:
                    for g in range(4):
                        nc.tensor.matmul(z1[0:128, g * 64:(g + 1) * 64],
                                         lhsT=w1h(g), rhs=h1[rd][0:H, :],
                                         start=False, stop=(g == 3))
                # cell2(t-1): open z2 with w2h*h2(t-2) (+b2 via ones row),
                # close with w2x*h1(t-1)
                if t > 0:
                    z2 = zpool2.tile([128, 512], _DT.float32, tag="z2")
                    for g in range(4):
                        nc.tensor.matmul(z2[0:128, g * 64:(g + 1) * 64],
                                         lhsT=w2h(g), rhs=h2[wr][0:H + 1, :],
                                         start=(g == 0), stop=False)
                    for g in range(4):
                        nc.tensor.matmul(z2[0:128, g * 64:(g + 1) * 64],
                                         lhsT=w2x(g), rhs=h1[rd][0:H, :],
                                         start=False, stop=(g == 3))

                # cell1(t) chain
                tg1 = wpool.tile([128, 256], _DT.bfloat16, tag="tg1")
                nc.scalar.activation(tg1[0:H, :], z1[0:H, 0:256], TANH)
                cell(tg1, c1, h1[wr], c2_on_pool=False)

                # cell2(t-1) shadow work
                if t > 0:
                    tg2 = wpool.tile([128, 256], _DT.bfloat16, tag="tg2")
                    nc.scalar.activation(tg2[0:H, :], z2[0:H, 0:256], TANH)
                    cell(tg2, c2, h2[rd], c2_on_pool=True)

                # open next step's z1 with the x-part
                if t + 1 < T:
                    z1 = zpool1.tile([128, 512], _DT.float32, tag="z1")
                    for g in range(4):
                        nc.tensor.matmul(z1[0:128, g * 64:(g + 1) * 64],
                                         lhsT=w1x(g),
                                         rhs=xt[0:H + 1, (t + 1) * BC:(t + 2) * BC],
                                         start=(g == 0), stop=False)

            # epilogue: cell2(T-1)
            last = (T - 1) % 2
            z2 = zpool2.tile([128, 512], _DT.float32, tag="z2")
            for g in range(4):
                nc.tensor.matmul(z2[0:128, g * 64:(g + 1) * 64],
                                 lhsT=w2h(g), rhs=h2[last ^ 1][0:H + 1, :],
                                 start=(g == 0), stop=False)
            for g in range(4):
                nc.tensor.matmul(z2[0:128, g * 64:(g + 1) * 64],
                                 lhsT=w2x(g), rhs=h1[last][0:H, :],
                                 start=False, stop=(g == 3))
            tg2 = wpool.tile([128, 256], _DT.bfloat16, tag="tg2")
            nc.scalar.activation(tg2[0:H, :], z2[0:H, 0:256], TANH)
            cell(tg2, c2, h2[last], c2_on_pool=False)

            # classifier head: pred = h2 @ (wf1 @ wf2) + fused bias
            predp = fpool.tile([128, BC], _DT.float32, tag="pred")
            nc.tensor.matmul(predp[0:NCLS, :], lhsT=wf12,
                             rhs=h2[last][0:H + 1, :], start=True, stop=True)
            nc.vector.tensor_copy(outs[:, :], predp[0:NCLS, :])
            nc.sync.dma_start(out=out_d[:, :], in_=outs[:, :])

    return nc


@functools.lru_cache(maxsize=1)
def _get_nc():
    return _build_nc()


def _scaled_gate_blocks(kmat, rows, extra_scale):
    """[rows x 512] tile: gate blocks reordered to [i,f,o,j], padded
    100->128 cols, prescaled for the tanh-only gate trick."""
    out = np.zeros((rows.stop - rows.start, 512), np.float32)
    for slot in range(4):
        b = SLOT_SRC_BLOCK[slot]
        out[:, slot * 128:slot * 128 + H] = (
            kmat[rows, b * H:(b + 1) * H] * (SLOT_PRESCALE[slot] * extra_scale))
    return out


def _prep_weights(k1, b1, k2, b2, w_fc1, b_fc1, w_fc2, b_fc2):
    w1x = np.zeros((H + 1, 512), np.float32)
    w1x[0:H] = _scaled_gate_blocks(k1, slice(0, H), 1.0)
    w2h = np.zeros((H + 1, 512), np.float32)
    w2h[0:H] = _scaled_gate_blocks(k2, slice(H, 2 * H), 0.5)
    for slot in range(4):
        b = SLOT_SRC_BLOCK[slot]
        fb = FORGET_BIAS if slot == 1 else 0.0
        w1x[H, slot * 128:slot * 128 + H] = (b1[b * H:(b + 1) * H] + fb) * SLOT_PRESCALE[slot]
        w2h[H, slot * 128:slot * 128 + H] = (b2[b * H:(b + 1) * H] + fb) * SLOT_PRESCALE[slot]
    w1h = _scaled_gate_blocks(k1, slice(H, 2 * H), 0.5)
    w2x = _scaled_gate_blocks(k2, slice(0, H), 0.5)
    # fused classifier: pred = h2 @ wf1 @ wf2 + (b_fc1 @ wf2 + b_fc2);
    # rows 0:H absorb the 0.5 for the doubled h~, row H rides the ones row
    wf12 = np.zeros((H + 1, NCLS), np.float32)
    wf12[0:H] = 0.5 * (w_fc1 @ w_fc2)
    wf12[H] = b_fc1 @ w_fc2 + b_fc2
    w1d = np.zeros((128, 1024), np.float32)
    w1d[0:H + 1, 0:512] = w1x
    w1d[0:H, 512:1024] = w1h
    w2d = np.zeros((128, 1026), np.float32)
    w2d[0:H, 0:512] = w2x
    w2d[0:H + 1, 512:1024] = w2h
    w2d[0:H + 1, 1024:1026] = wf12
    return {"w1d": w1d.astype(BF16), "w2d": w2d.astype(BF16)}


def _run(inputs, trace=False):
    nc = _get_nc()
    feats = np.asarray(inputs["features"])
    x = np.asarray(inputs["embedding"])[feats]          # [B, T, H] host gather
    shared = _prep_weights(
        np.asarray(inputs["k1"]), np.asarray(inputs["b1"]),
        np.asarray(inputs["k2"]), np.asarray(inputs["b2"]),
        np.asarray(inputs["w_fc1"]), np.asarray(inputs["b_fc1"]),
        np.asarray(inputs["w_fc2"]), np.asarray(inputs["b_fc2"]))
    in_maps = []
    for c in range(N_CORES):
        xtd = np.zeros((128, T * BC), np.float32)
        # [BC, T, H] -> [H, T*BC] feature-major; partition 100 = bias ones
        xtd[0:H] = x[c * BC:(c + 1) * BC].transpose(2, 1, 0).reshape(H, T * BC)
        xtd[H] = 1.0
        in_maps.append({**shared, "xtd": xtd.astype(BF16)})
    res = run_bass_kernel_spmd(nc, in_maps, core_ids=list(range(N_CORES)),
                               trace=trace)
    out = np.empty((B, NCLS), np.float32)
    for c in range(N_CORES):
        out[c * BC:(c + 1) * BC] = res.results[c]["out"].T
    return out, res


def kernel(**inputs):
    out, _ = _run(inputs, trace=False)
    return out


# revision 10
# speedup vs baseline: 1.7217x; 1.7217x over previous
"""Trainium2 Bass kernel for a 2-layer LSTM text classifier.

Model (see original nn.Module): embedding lookup -> 2-layer BasicLSTM
(H=100, T=25) -> dense(128) -> dense(2). Batch 512 is data-parallel
across 8 NeuronCores (64 rows/core); all parameters are replicated.
The embedding gather (pure indexing) happens host-side so only the
5 MB of used rows ship to the devices, pre-transposed to the
feature-major layout the kernel wants.

Device kernel design notes (v2 — latency-tuned):
- Feature-major layout everywhere: [hidden=100 partitions, batch=64
  free]; the recurrence never needs a transpose.
- All four gates go through a single tanh activation per cell:
  sigmoid(x) = (1 + tanh(x/2))/2, with the 1/2 prescale folded into
  the i/f/o weight columns host-side. Cell state is kept doubled
  (C = 2c):
      qh = (tf + 1) * C           # = 4 c sigmoid(f+1)
      ph = (ti + 1) * tj          # = 2 sigmoid(i) tanh(j)
      C' = qh * 0.5 + ph          # = 2 c'
      tc = tanh(0.5 * C')         # = tanh(c')
      h~ = (to + 1) * tc          # = 2 h   (next-layer weights absorb 0.5)
- Software pipeline: phase t computes cell1(t) and cell2(t-1). The
  critical chain is cell1's h-recurrence: z1close mms -> gates tanh
  (ACT) -> qh -> C' (DVE) -> tc (ACT) -> h (DVE). Everything of
  cell2 runs in that chain's engine gaps one phase behind.
- ph rides on GpSimd (off the DVE chain), and cell2's C' also runs
  on GpSimd so cell1's h op never queues behind it on DVE.
- z2's PSUM bank opens with the w2h*h2(t-2) part (ready early) and
  closes with w2x*h1(t-1) (the same sem that starts the chain).
- Biases ride in an extra contraction row: xt carries a row of ones
  at partition 100 (b1 in w1x row 100); h2 tiles carry a ones row
  (b2 in w2h row 100); the fused classifier wf12 = wf1 @ wf2 + bias
  rides the same way.
- All inputs are pre-transposed on the host into [128, N] layouts so
  the loads are plain direct DMAs (contiguous >=1KB per partition
  line) split across the SP/ACT/DVE HWDGE rings in first-use order.
- Matmuls are bf16 accumulating in fp32 PSUM; cell state stays fp32.
"""

import functools
import os
import sys

import numpy as np

for _p in ("/opt/trn_rl_repo", "/root/.axon_site/_ro/trn_rl_repo"):
    if os.path.isdir(_p) and _p not in sys.path:
        sys.path.insert(0, _p)
        break

import ml_dtypes

from concourse import bass, bass2jax, mybir
from concourse.bass_utils import run_bass_kernel_spmd
from concourse.tile import TileContext

# --- BIR sync-wait rebalancer -------------------------------------------
# The walrus build in this image enforces ONE sync-wait command per ISA
# instruction struct, but Tile's semaphore assignment happily emits 2-4
# waits on matmuls/DVE ops at psum-recycle points. Rewrite the BIR before
# walrus: park one matmul wait on the adjacent Ldweights (same in-order
# queue, executes strictly before the matmul) and split any remaining
# excess onto pure-wait EventSemaphore carriers inserted directly before
# the offending instruction on its own queue. Semantics are unchanged --
# every wait still completes before the instruction it guarded.

_WAIT_PASSTHROUGH = {"EventSemaphore", "UnconditionalBranch", "Call",
                     "RegisterMove", "ISA"}


def _rebalance_bir_waits(bir_bytes):
    import orjson
    bir = orjson.loads(bir_bytes)
    n = 0
    for fn in bir["functions"]:
        for blk in fn["blocks"]:
            out = []
            prev = None
            for inst in blk["instructions"]:
                op = inst.get("opcode")
                si = inst.get("sync_info") or {}
                waits = si.get("on_wait") or []
                if op not in _WAIT_PASSTHROUGH and len(waits) > 1:
                    if (op == "Matmult" and prev is not None
                            and prev.get("opcode") == "Ldweights"
                            and not (prev.get("sync_info") or {}).get("on_wait")):
                        tsi = prev.setdefault("sync_info", {})
                        tsi.setdefault("on_wait", []).append(waits.pop(0))
                    while len(waits) > 1:
                        n += 1
                        out.append({
                            "debug": inst.get("debug", 0),
                            "engine": inst["engine"],
                            "ins": [], "outs": [],
                            "name": f"antwait_{n}",
                            "opcode": "EventSemaphore",
                            "sync_info": {"on_update": [],
                                          "on_wait": [waits.pop(0)]},
                        })
                    si["on_wait"] = waits
                out.append(inst)
                prev = inst
            blk["instructions"] = out
    return orjson.dumps(bir)


_orig_compile_bir_kernel = bass2jax.compile_bir_kernel


def _compile_bir_kernel_rebalanced(bir_json, tmpdir, neff_name="file.neff"):
    return _orig_compile_bir_kernel(_rebalance_bir_waits(bir_json), tmpdir,
                                    neff_name=neff_name)


if bass2jax.compile_bir_kernel is not _compile_bir_kernel_rebalanced:
    bass2jax.compile_bir_kernel = _compile_bir_kernel_rebalanced

H = 100          # hidden size
T = 25           # sequence length
B = 512          # total batch
N_CORES = 8
BC = B // N_CORES  # 64 per-core batch
NCLS = 2         # logits
FORGET_BIAS = 1.0

BF16 = ml_dtypes.bfloat16
_DT = mybir.dt
TANH = mybir.ActivationFunctionType.Tanh
ADD = mybir.AluOpType.add
MULT = mybir.AluOpType.mult

# gate slot order in PSUM: [i, f, o, j]; source block order in the
# TF BasicLSTMCell kernel is [i, j, f, o]
SLOT_SRC_BLOCK = (0, 2, 3, 1)
SLOT_PRESCALE = (0.5, 0.5, 0.5, 1.0)  # tanh(x/2) trick for i/f/o, plain tanh for j


def _build_nc():
    nc = bass.Bass()
    # host-pretransposed inputs: partition-major, contiguous per row
    w1d = nc.dram_tensor("w1d", [128, 1024], _DT.bfloat16, kind="ExternalInput")
    w2d = nc.dram_tensor("w2d", [128, 1026], _DT.bfloat16, kind="ExternalInput")
    xtd = nc.dram_tensor("xtd", [128, T * BC], _DT.bfloat16, kind="ExternalInput")
    out_d = nc.dram_tensor("out", [NCLS, BC], _DT.float32, kind="ExternalOutput")

    with TileContext(nc) as tc:
        with tc.tile_pool(name="const", bufs=1) as cpool, \
             tc.tile_pool(name="work", bufs=3) as wpool, \
             tc.tile_pool(name="ps1", bufs=2, space="PSUM") as zpool1, \
             tc.tile_pool(name="ps2", bufs=2, space="PSUM") as zpool2, \
             tc.tile_pool(name="psfc", bufs=1, space="PSUM") as fpool:

            wp = cpool.tile([128, 2052], _DT.bfloat16, tag="wp")
            xt = cpool.tile([128, T * BC], _DT.bfloat16, tag="xt")
            hst = cpool.tile([128, 4 * BC], _DT.bfloat16, tag="hst")
            cst = cpool.tile([128, 2 * BC], _DT.float32, tag="cst")
            scratch = cpool.tile([1, 1], _DT.float32, tag="scratch")
            outs = cpool.tile([NCLS, BC], _DT.float32, tag="outs")

            # weight slices (lhsT layout: partitions = contraction dim)
            def w1x(g):
                return wp[0:H + 1, g * 128:(g + 1) * 128]

            def w1h(g):
                return wp[0:H, 512 + g * 128:512 + (g + 1) * 128]

            def w2x(g):
                return wp[0:H, 1024 + g * 128:1024 + (g + 1) * 128]

            def w2h(g):
                return wp[0:H + 1, 1536 + g * 128:1536 + (g + 1) * 128]

            wf12 = wp[0:H + 1, 2048:2050]

            # recurrent state: ping/pong columns; hst partition 100 is the
            # all-ones bias lane (used by the h2-side w2h matmuls + head)
            h1 = [hst[:, 0:BC], hst[:, BC:2 * BC]]
            h2 = [hst[:, 2 * BC:3 * BC], hst[:, 3 * BC:4 * BC]]
            c1 = cst[:, 0:BC]
            c2 = cst[:, BC:2 * BC]

            # input DMAs, split across HWDGE rings in first-use order
            nc.sync.dma_start(out=wp[:, 0:1024], in_=w1d[:, :])
            nc.scalar.dma_start(out=xt[:, 0:8 * BC], in_=xtd[:, 0:8 * BC])
            nc.scalar.dma_start(out=wp[:, 1024:2050], in_=w2d[:, :])
            nc.sync.dma_start(out=xt[:, 8 * BC:T * BC], in_=xtd[:, 8 * BC:T * BC])

            # state init + tanh table warm-up; VEC is idle this early, and
            # GpSimd rejects partition ranges off its 16-partition grid
            nc.vector.memset(scratch[:, :], 0.0)
            nc.scalar.activation(scratch[:, :], scratch[:, :], TANH)
            nc.vector.memset(cst[:, :], 0.0)
            # partition offsets must be 32-aligned, so build the ones row at
            # partition 100 by filling with 1.0 then zeroing rows 0:100
            nc.vector.memset(hst[:, :], 1.0)
            nc.vector.memset(hst[0:H, :], 0.0)

            def cell(tg, c_st, h_wr, c2_on_pool):
                # gates -> new cell state / hidden, [H, BC] layout. Tiles
                # touched by GpSimd span all 128 partitions (Pool accesses
                # must be 16-partition aligned); rows 100-127 carry garbage
                # that no consumer reads.
                ti, tf = tg[:, 0:64], tg[:, 64:128]
                to, tj = tg[0:H, 128:192], tg[:, 192:256]
                qh = wpool.tile([128, BC], _DT.float32, tag="qh")
                nc.vector.scalar_tensor_tensor(qh[:, :], tf, 1.0, c_st, op0=ADD, op1=MULT)
                ph = wpool.tile([128, BC], _DT.bfloat16, tag="ph")
                nc.vector.scalar_tensor_tensor(ph[:, :], ti, 1.0, tj, op0=ADD, op1=MULT)
                nc.vector.scalar_tensor_tensor(c_st, qh[:, :], 0.5, ph[:, :], op0=MULT, op1=ADD)
                tcg = wpool.tile([H, BC], _DT.bfloat16, tag="tc")
                nc.scalar.activation(tcg[:, :], c_st[0:H, :], TANH, scale=0.5)
                nc.vector.scalar_tensor_tensor(h_wr[0:H, :], to, 1.0, tcg[:, :], op0=ADD, op1=MULT)

            # prologue: open z1(0) with the x-part (h1(-1)=0, so this also
            # closes it -- the recurrent term is exactly zero at t=0)
            z1 = zpool1.tile([128, 512], _DT.float32, tag="z1")
            for g in range(4):
                nc.tensor.matmul(z1[0:128, g * 64:(g + 1) * 64],
                                 lhsT=w1x(g), rhs=xt[0:H + 1, 0:BC],
                                 start=(g == 0), stop=(g == 3))

            z2 = None
            for t in range(T):
                rd, wr = (t + 1) % 2, t % 2
                # close z1(t) with the recurrent part (the chain head)
                if t > 0:
                    for g in range(4):
                        nc.tensor.matmul(z1[0:128, g * 64:(g + 1) * 64],
                                         lhsT=w1h(g), rhs=h1[rd][0:H, :],
                                         start=False, stop=(g == 3))
                # cell2(t-1): open z2 with w2h*h2(t-2) (+b2 via ones row),
                # close with w2x*h1(t-1)
                if t > 0:
                    z2 = zpool2.tile([128, 512], _DT.float32, tag="z2")
                    for g in range(4):
                        nc.tensor.matmul(z2[0:128, g * 64:(g + 1) * 64],
                                         lhsT=w2h(g), rhs=h2[wr][0:H + 1, :],
                                         start=(g == 0), stop=False)
                    for g in range(4):
                        nc.tensor.matmul(z2[0:128, g * 64:(g + 1) * 64],
                                         lhsT=w2x(g), rhs=h1[rd][0:H, :],
                                         start=False, stop=(g == 3))

                # cell1(t) chain
                tg1 = wpool.tile([128, 256], _DT.bfloat16, tag="tg1")
                nc.scalar.activation(tg1[0:H, :], z1[0:H, 0:256], TANH)
                cell(tg1, c1, h1[wr], c2_on_pool=False)

                # cell2(t-1) shadow work
                if t > 0:
                    tg2 = wpool.tile([128, 256], _DT.bfloat16, tag="tg2")
                    nc.scalar.activation(tg2[0:H, :], z2[0:H, 0:256], TANH)
                    cell(tg2, c2, h2[rd], c2_on_pool=True)

                # open next step's z1 with the x-part
                if t + 1 < T:
                    z1 = zpool1.tile([128, 512], _DT.float32, tag="z1")
                    for g in range(4):
                        nc.tensor.matmul(z1[0:128, g * 64:(g + 1) * 64],
                                         lhsT=w1x(g),
                                         rhs=xt[0:H + 1, (t + 1) * BC:(t + 2) * BC],
                                         start=(g == 0), stop=False)

            # epilogue: cell2(T-1)
            last = (T - 1) % 2
            z2 = zpool2.tile([128, 512], _DT.float32, tag="z2")
            for g in range(4):
                nc.tensor.matmul(z2[0:128, g * 64:(g + 1) * 64],
                                 lhsT=w2h(g), rhs=h2[last ^ 1][0:H + 1, :],
                                 start=(g == 0), stop=False)
            for g in range(4):
                nc.tensor.matmul(z2[0:128, g * 64:(g + 1) * 64],
                                 lhsT=w2x(g), rhs=h1[last][0:H, :],
                                 start=False, stop=(g == 3))
            tg2 = wpool.tile([128, 256], _DT.bfloat16, tag="tg2")
            nc.scalar.activation(tg2[0:H, :], z2[0:H, 0:256], TANH)
            cell(tg2, c2, h2[last], c2_on_pool=False)

            # classifier head: pred = h2 @ (wf1 @ wf2) + fused bias
            predp = fpool.tile([128, BC], _DT.float32, tag="pred")
            nc.tensor.matmul(predp[0:NCLS, :], lhsT=wf12,
                             rhs=h2[last][0:H + 1, :], start=True, stop=True)
            nc.vector.tensor_copy(outs[:, :], predp[0:NCLS, :])
            nc.sync.dma_start(out=out_d[:, :], in_=outs[:, :])

    return nc


@functools.lru_cache(maxsize=1)
def _get_nc():
    return _build_nc()


def _scaled_gate_blocks(kmat, rows, extra_scale):
    """[rows x 512] tile: gate blocks reordered to [i,f,o,j], padded
    100->128 cols, prescaled for the tanh-only gate trick."""
    out = np.zeros((rows.stop - rows.start, 512), np.float32)
    for slot in range(4):
        b = SLOT_SRC_BLOCK[slot]
        out[:, slot * 128:slot * 128 + H] = (
            kmat[rows, b * H:(b + 1) * H] * (SLOT_PRESCALE[slot] * extra_scale))
    return out


def _prep_weights(k1, b1, k2, b2, w_fc1, b_fc1, w_fc2, b_fc2):
    w1x = np.zeros((H + 1, 512), np.float32)
    w1x[0:H] = _scaled_gate_blocks(k1, slice(0, H), 1.0)
    w2h = np.zeros((H + 1, 512), np.float32)
    w2h[0:H] = _scaled_gate_blocks(k2, slice(H, 2 * H), 0.5)
    for slot in range(4):
        b = SLOT_SRC_BLOCK[slot]
        fb = FORGET_BIAS if slot == 1 else 0.0
        w1x[H, slot * 128:slot * 128 + H] = (b1[b * H:(b + 1) * H] + fb) * SLOT_PRESCALE[slot]
        w2h[H, slot * 128:slot * 128 + H] = (b2[b * H:(b + 1) * H] + fb) * SLOT_PRESCALE[slot]
    w1h = _scaled_gate_blocks(k1, slice(H, 2 * H), 0.5)
    w2x = _scaled_gate_blocks(k2, slice(0, H), 0.5)
    # fused classifier: pred = h2 @ wf1 @ wf2 + (b_fc1 @ wf2 + b_fc2);
    # rows 0:H absorb the 0.5 for the doubled h~, row H rides the ones row
    wf12 = np.zeros((H + 1, NCLS), np.float32)
    wf12[0:H] = 0.5 * (w_fc1 @ w_fc2)
    wf12[H] = b_fc1 @ w_fc2 + b_fc2
    w1d = np.zeros((128, 1024), np.float32)
    w1d[0:H + 1, 0:512] = w1x
    w1d[0:H, 512:1024] = w1h
    w2d = np.zeros((128, 1026), np.float32)
    w2d[0:H, 0:512] = w2x
    w2d[0:H + 1, 512:1024] = w2h
    w2d[0:H + 1, 1024:1026] = wf12
    return {"w1d": w1d.astype(BF16), "w2d": w2d.astype(BF16)}


def _run(inputs, trace=False):
    nc = _get_nc()
    feats = np.asarray(inputs["features"])
    x = np.asarray(inputs["embedding"])[feats]          # [B, T, H] host gather
    shared = _prep_weights(
        np.asarray(inputs["k1"]), np.asarray(inputs["b1"]),
        np.asarray(inputs["k2"]), np.asarray(inputs["b2"]),
        np.asarray(inputs["w_fc1"]), np.asarray(inputs["b_fc1"]),
        np.asarray(inputs["w_fc2"]), np.asarray(inputs["b_fc2"]))
    in_maps = []
    for c in range(N_CORES):
        xtd = np.zeros((128, T * BC), np.float32)
        # [BC, T, H] -> [H, T*BC] feature-major; partition 100 = bias ones
        xtd[0:H] = x[c * BC:(c + 1) * BC].transpose(2, 1, 0).reshape(H, T * BC)
        xtd[H] = 1.0
        in_maps.append({**shared, "xtd": xtd.astype(BF16)})
    res = run_bass_kernel_spmd(nc, in_maps, core_ids=list(range(N_CORES)),
                               trace=trace)
    out = np.empty((B, NCLS), np.float32)
    for c in range(N_CORES):
        out[c * BC:(c + 1) * BC] = res.results[c]["out"].T
    return out, res


def kernel(**inputs):
    out, _ = _run(inputs, trace=False)
    return out
